# revision 35
# baseline (speedup 1.0000x reference)
"""Trainium2 Bass kernel for nn_DeepCrossNetworkModel_Controller_hard.

Model: per-field embedding gather -> BatchNorm1d(F) (eval) -> controller
linear + softmax over fields -> top-k mask (renormalized) -> CrossNetwork(6)
+ MLP(2496->1024->512, BN+ReLU) -> concat -> linear -> sigmoid.

Strategy (data-parallel over 8 NeuronCores, 2048 rows each):
 - BN folded into the embedding table on host; table stored bf16 padded to
   128 elems/row so dma_gather(transpose=True) lands embeddings
   FEATURE-major directly (no PE transposes at all).
 - top-k of softmax + renormalize == softmax restricted to top-k logits.
 - CrossNetwork collapses algebraically: on device only U = x0 @ wu
   (7 cols, folded into the MLP0 m-tile loop) + a scalar recursion.
 - Software-pipelined: gathers run two blocks ahead, controller/top-k of
   block b+1 execute under MLP0 of block b, so the PE never idles.
"""

import sys

if "/opt/trn_rl_repo" not in sys.path:
    sys.path.insert(0, "/opt/trn_rl_repo")

import ml_dtypes
import numpy as np

import concourse.bass as bass
import concourse.bacc as bacc
import concourse.mybir as mybir
import concourse.tile as tile
from concourse.bass_utils import run_bass_kernel_spmd
from concourse.masks import make_identity

# Problem constants (hardcoded per spec).
B, F, E, L = 16384, 39, 64, 6
VOCAB = 10000
D = F * E  # 2496
H0, H1 = 1024, 512
EPS = 1e-5
NCORES = 8
BPC = B // NCORES      # 2048 rows per core
BLK = 512              # batch block
NBLK = BPC // BLK      # 4
NCHUNK = BLK // 128    # 4 chunks of 128 rows per block
KT = 20                # feature k-tiles of 128 (D padded 2496 -> 2560)
M0 = H0 // 128         # 8
M1 = H1 // 128         # 4
KT1 = H0 // 128        # 8
NQ = 4                 # SWDGE queues
# idx widths (wrapped-by-16 free dims): 39 single-field gathers of 512
# (transpose-mode dma_gather hangs on HW above 512 idxs per call)
IDXW_F = 512 // 16       # 32
IDXW_TOT = 39 * IDXW_F   # 1248: 20 even-field (direct) + 19 odd (scratch)

dt = mybir.dt
AF = mybir.ActivationFunctionType
OP = mybir.AluOpType
bf16 = ml_dtypes.bfloat16

_CACHE = {}


def _build(k, v_consts, c0, queue_map=None, ablate=None, all_scratch=False,
           sp=True):
    """Build the per-core SPMD bass module.

    queue_map: optional {(blk, g): queue_num} overriding the default
    g % NQ assignment, used to realign SWDGE queues with the
    scheduler's mod-8 DMASW semaphore rotation (see _build_aligned).
    """
    queue_map = queue_map or {}
    gather_insts = {}
    nc = bacc.Bacc("TRN2", target_bir_lowering=False, debug=False,
                   num_devices=NCORES, num_swdge_queues=NQ)
    nc._gather_insts = gather_insts

    idxs_d = nc.declare_dram_parameter("idxs", [NBLK, 128, IDXW_TOT], dt.int16, isOutput=False)
    tab_d = nc.declare_dram_parameter("tab", [F * VOCAB, 128], dt.bfloat16, isOutput=False)
    wc_d = nc.declare_dram_parameter("wc", [128, KT * F], dt.bfloat16, isOutput=False)
    w0_d = nc.declare_dram_parameter("w0", [128, KT * M0 * 128], dt.bfloat16, isOutput=False)
    w1_d = nc.declare_dram_parameter("w1", [128, KT1 * M1 * 128], dt.bfloat16, isOutput=False)
    wu_d = nc.declare_dram_parameter("wu", [128, KT * 7], dt.bfloat16, isOutput=False)
    s_d = nc.declare_dram_parameter("s", [F, KT * 128], dt.bfloat16, isOutput=False)
    lw2_d = nc.declare_dram_parameter("lw2", [128, M1], dt.bfloat16, isOutput=False)
    b0_d = nc.declare_dram_parameter("b0", [128, M0], dt.float32, isOutput=False)
    b1_d = nc.declare_dram_parameter("b1", [128, M1], dt.float32, isOutput=False)
    out_d = nc.declare_dram_parameter("out", [BPC], dt.float32, isOutput=True)
    if ablate == "dumpflat":
        fdump_d = nc.declare_dram_parameter(
            "fdump", [NBLK, 128, KT * BLK], dt.bfloat16, isOutput=True)

    rounds = -(-k // 8)  # ceil(k/8) match_replace rounds

    with tile.TileContext(nc) as tc:
        with (
            tc.tile_pool(name="const", bufs=1) as cpool,
            tc.tile_pool(name="flat", bufs=1) as flatp,
            tc.tile_pool(name="big", bufs=1) as bigp,
            tc.tile_pool(name="msk", bufs=2) as mskp,
            tc.tile_pool(name="gat", bufs=20) as gatp,
            tc.tile_pool(name="idx", bufs=3) as idxp,
            tc.tile_pool(name="scr", bufs=8) as scr,
            # PSUM slots are bank-granular (8 banks x 2KB/partition):
            # psb "b" x3 (hp/ex/up) + psz "z" x2 + pss "s" x2 + pmt "t" x1
            tc.tile_pool(name="psb", bufs=3, space="PSUM") as psb,
            tc.tile_pool(name="psz", bufs=2, space="PSUM") as psz,
            tc.tile_pool(name="pss", bufs=2, space="PSUM") as pss,
            tc.tile_pool(name="pmt", bufs=1, space="PSUM") as pmt,
        ):
            # ---- identities + small constants first (cheap, unblock PE) ----
            idf = cpool.tile([128, 128], dt.float32)
            make_identity(nc, idf[:])
            idb = cpool.tile([128, 128], dt.bfloat16)
            make_identity(nc, idb[:])

            wc_sb = cpool.tile([128, KT * F], dt.bfloat16)
            nc.sync.dma_start(wc_sb[:], wc_d[:])
            s_sb = cpool.tile([F, KT * 128], dt.bfloat16)
            nc.sync.dma_start(s_sb[:], s_d[:])
            wu_sb = cpool.tile([128, KT * 7], dt.bfloat16)
            nc.sync.dma_start(wu_sb[:], wu_d[:])
            lw2_sb = cpool.tile([128, M1], dt.bfloat16)
            nc.sync.dma_start(lw2_sb[:], lw2_d[:])
            b0_sb = cpool.tile([128, M0], dt.float32)
            nc.sync.dma_start(b0_sb[:], b0_d[:])
            b1_sb = cpool.tile([128, M1], dt.float32)
            nc.sync.dma_start(b1_sb[:], b1_d[:])
            # big weights last, in pieces, so early DMA traffic unblocks first
            w1_sb = cpool.tile([128, KT1 * M1 * 128], dt.bfloat16)
            nc.sync.dma_start(w1_sb[:], w1_d[:])
            w0_sb = cpool.tile([128, KT * M0 * 128], dt.bfloat16)
            for q in range(4):
                qs = KT * M0 * 128 // 4
                nc.sync.dma_start(w0_sb[:, q * qs : (q + 1) * qs],
                                  w0_d[:, q * qs : (q + 1) * qs])

            # ---- persistent activations ----
            # flat/x0 triple buffer: landing b+2, ctrl reads b+1, x0 in b
            flat_fm = [flatp.tile([128, KT * BLK], dt.bfloat16, tag=f"f{i}",
                                  name=f"flat{i}")
                       for i in range(3)]
            h0_fm = bigp.tile([128, M0 * BLK], dt.bfloat16)
            h1_fm = bigp.tile([128, M1 * BLK], dt.bfloat16)
            p_sb = bigp.tile([128, BPC // 128], dt.float32)

            nreg = nc.gpsimd.to_reg(512)
            qcount = [0]  # global SWDGE queue round-robin

            idx_sb = {}
            gts = {}
            masks = {}
            zs = {}
            als = {}

            def emit_idx_dma(blk):
                t = idxp.tile([128, IDXW_TOT], dt.int16, tag="i")
                nc.sync.dma_start(t[:], idxs_d[blk, :, :])
                idx_sb[blk] = t

            def _gather(key, out_ap, lo, it, slot):
                inst = nc.gpsimd.dma_gather(
                    out_ap=out_ap,
                    in_ap=tab_d[lo : lo + VOCAB, :],
                    idxs_ap=it[:, slot * IDXW_F : (slot + 1) * IDXW_F],
                    num_idxs=512,
                    num_idxs_reg=nreg,
                    elem_size=128,
                    transpose=True,
                    single_packet=sp,
                    queue_num=queue_map.get(key, qcount[0] % NQ),
                )
                qcount[0] += 1
                gather_insts[inst.ins.name] = key

            def emit_gathers(blk):
                """39 single-field 512-idx gathers per block: even fields
                land directly in flat k-tile slots (with zeros on parts
                64:128), odd fields go to scratch for a DVE merge."""
                it = idx_sb.pop(blk)
                ft = flat_fm[blk % 3]
                for g in range(KT):
                    f = min(2 * g, 38)
                    if all_scratch:
                        etile = gatp.tile([128, BLK], dt.bfloat16, tag="e",
                                          name="etile")
                        _gather((blk, g, 0),
                                etile[:].rearrange("p (a n) -> p a n", a=1),
                                f * VOCAB, it, g)
                        gts[(blk, g, 0)] = etile
                    else:
                        _gather((blk, g, 0),
                                ft[:, g * BLK : (g + 1) * BLK].rearrange(
                                    "p (a n) -> p a n", a=1),
                                f * VOCAB, it, g)
                    if g < 19:
                        gtile = gatp.tile([128, BLK], dt.bfloat16, tag="g")
                        _gather((blk, g, 1),
                                gtile[:].rearrange("p (a n) -> p a n", a=1),
                                (2 * g + 1) * VOCAB, it, KT + g)
                        gts[(blk, g)] = gtile

            def emit_merges(blk):
                """DVE copies moving odd-field scratch into flat upper
                partitions, plus the ones-row bias feature."""
                ft = flat_fm[blk % 3]
                for g in range(KT):
                    if all_scratch:
                        etile = gts.pop((blk, g, 0))
                        nc.vector.tensor_copy(
                            ft[0:128, g * BLK : (g + 1) * BLK],
                            etile[0:128, 0:BLK])
                for g in range(19):
                    gtile = gts.pop((blk, g))
                    nc.vector.tensor_copy(
                        ft[64:128, g * BLK : (g + 1) * BLK],
                        gtile[0:64, 0:BLK])
                nc.vector.memset(ft[64:65, 19 * BLK : 20 * BLK], 1.0)

            def emit_ctrl(blk):
                """Controller matmuls: z[chunk] = flat_chunk^T @ wc.
                All 4 chunks share one PSUM bank tile (col ranges)."""
                ft = flat_fm[blk % 3]
                z_all = psz.tile([128, 256], dt.float32, space="PSUM", tag="z")
                for c in range(NCHUNK):
                    for kt in range(KT):
                        nc.tensor.matmul(
                            z_all[:, c * 64 : c * 64 + F],
                            lhsT=ft[:, kt * BLK + c * 128 : kt * BLK + (c + 1) * 128],
                            rhs=wc_sb[:, kt * F : (kt + 1) * F],
                            start=(kt == 0), stop=(kt == KT - 1),
                        )
                zs[blk] = z_all

            def emit_topk(blk):
                """Top-k + renormalized softmax -> batch-major mask (bf16)."""
                z_all = zs.pop(blk)
                for c in range(NCHUNK):
                    z = z_all[:, c * 64 : (c + 1) * 64]
                    mx = scr.tile([128, 8], dt.float32, tag="mx")
                    nm = scr.tile([128, 1], dt.float32, tag="nm")
                    zap = scr.tile([128, F], dt.float32, tag="zap")
                    zap2 = scr.tile([128, F], dt.float32, tag="zap2")
                    esb = scr.tile([128, F], dt.float32, tag="esb")
                    ssum = scr.tile([128, 1], dt.float32, tag="ssum")
                    rcp = scr.tile([128, 1], dt.float32, tag="rcp")
                    mbm = scr.tile([128, F], dt.bfloat16, tag="mbm")
                    src = z[:, :F]
                    outs = [zap[:], zap2[:]]
                    for r in range(rounds):
                        nc.vector.max(out=mx[:], in_=src)
                        if r == 0:
                            nc.vector.tensor_scalar(
                                nm[:], mx[:, 0:1], -1.0, None, op0=OP.mult)
                        if r == rounds - 1 and k - 8 * r < 8:
                            nc.vector.memset(mx[:, k - 8 * r :], -1e30)
                        nc.vector.match_replace(
                            out=outs[r % 2], in_to_replace=mx[:],
                            in_values=src, imm_value=-1e30)
                        src = outs[r % 2]
                    zfin = outs[(rounds - 1) % 2]
                    nc.scalar.activation(esb[:], z[:, :F], AF.Exp,
                                         bias=nm[:, 0:1], scale=1.0)
                    nc.vector.tensor_scalar(zfin, zfin, -1e30, None,
                                            op0=OP.is_equal)
                    nc.vector.tensor_tensor(esb[:], esb[:], zfin, op=OP.mult)
                    nc.vector.reduce_sum(ssum[:], esb[:],
                                         axis=mybir.AxisListType.X)
                    nc.vector.reciprocal(rcp[:], ssum[:])
                    nc.vector.tensor_scalar(mbm[:], esb[:], rcp[:, 0:1],
                                            None, op0=OP.mult)
                    masks[(blk, c)] = mbm

            def emit_mask_transpose(blk):
                """Batch-major masks -> feature-major mask_fm [F, BLK]."""
                mt_fm = mskp.tile([F, BLK], dt.bfloat16, tag="m")
                mt = pmt.tile([128, BLK], dt.bfloat16, space="PSUM", tag="t")
                for c in range(NCHUNK):
                    mbm = masks.pop((blk, c))
                    nc.tensor.transpose(
                        out=mt[:F, c * 128 : (c + 1) * 128], in_=mbm[:],
                        identity=idb[:])
                    nc.vector.tensor_copy(
                        mt_fm[:, c * 128 : (c + 1) * 128],
                        mt[:F, c * 128 : (c + 1) * 128])
                masks[blk] = mt_fm

            def emit_expand_x0(blk):
                """Expand mask over features (PE) and apply to flat in
                place: x0 = flat * S^T mask."""
                ft = flat_fm[blk % 3]
                mt_fm = masks.pop(blk)
                for kt in range(KT):
                    ex = psb.tile([128, BLK], dt.float32, space="PSUM",
                                  tag="b")
                    nc.tensor.matmul(
                        ex[:], lhsT=s_sb[:, kt * 128 : (kt + 1) * 128],
                        rhs=mt_fm[:], start=True, stop=True)
                    nc.vector.tensor_tensor(
                        ft[:, kt * BLK : (kt + 1) * BLK],
                        ft[:, kt * BLK : (kt + 1) * BLK], ex[:],
                        op=OP.mult)

            def emit_mlp0_u(blk):
                """U (7-wide m-tile) first, then MLP0's 8 m-tiles."""
                ft = flat_fm[blk % 3]
                up = psb.tile([128, BLK], dt.float32, space="PSUM", tag="b")
                for kt in range(KT):
                    nc.tensor.matmul(
                        up[:7, :],
                        lhsT=wu_sb[:, kt * 7 : (kt + 1) * 7],
                        rhs=ft[:, kt * BLK : (kt + 1) * BLK],
                        start=(kt == 0), stop=(kt == KT - 1),
                    )
                u_sb = scr.tile([7, BLK], dt.float32, tag="usb", bufs=2)
                nc.vector.tensor_copy(u_sb[:], up[:7, :])
                for m in range(M0):
                    hp = psb.tile([128, BLK], dt.float32, space="PSUM",
                                  tag="b")
                    for kt in range(KT):
                        nc.tensor.matmul(
                            hp[:],
                            lhsT=w0_sb[:, (kt * M0 + m) * 128 : (kt * M0 + m + 1) * 128],
                            rhs=ft[:, kt * BLK : (kt + 1) * BLK],
                            start=(kt == 0), stop=(kt == KT - 1),
                        )
                    nc.scalar.activation(h0_fm[:, m * BLK : (m + 1) * BLK],
                                         hp[:], AF.Relu,
                                         bias=b0_sb[:, m : m + 1], scale=1.0)
                return u_sb

            def emit_u_alpha(blk, u_sb):
                """Transpose u to batch-major and run the cross-collapse
                scalar recursion per 128-row chunk."""
                ut_all = pss.tile([128, 256], dt.float32, space="PSUM",
                                  tag="s")
                for c in range(NCHUNK):
                    nc.tensor.transpose(
                        out=ut_all[:, c * 64 : c * 64 + 7],
                        in_=u_sb[:, c * 128 : (c + 1) * 128],
                        identity=idf[:7, :7],
                    )
                    usb = scr.tile([128, 8], dt.float32, tag="ut")
                    nc.vector.tensor_copy(usb[:, :7],
                                          ut_all[:, c * 64 : c * 64 + 7])
                    al = scr.tile([128, 1], dt.float32, tag="al")
                    t1 = scr.tile([128, 1], dt.float32, tag="t1")
                    nc.vector.tensor_scalar(al[:], usb[:, 0:1],
                                            1.0 + v_consts[0], None, op0=OP.add)
                    for l in range(1, L):
                        nc.vector.tensor_scalar(t1[:], usb[:, l : l + 1],
                                                1.0, None, op0=OP.add)
                        nc.vector.tensor_tensor(al[:], al[:], t1[:],
                                                op=OP.mult)
                        if v_consts[l] != 0.0:
                            nc.vector.tensor_scalar(al[:], al[:],
                                                    v_consts[l], None,
                                                    op0=OP.add)
                    nc.vector.tensor_tensor(al[:], al[:], usb[:, 6:7],
                                            op=OP.mult)
                    als[(blk, c)] = al

            def emit_mlp1(blk):
                for m in range(M1):
                    hp = psb.tile([128, BLK], dt.float32, space="PSUM",
                                  tag="b")
                    for kt in range(KT1):
                        nc.tensor.matmul(
                            hp[:],
                            lhsT=w1_sb[:, (kt * M1 + m) * 128 : (kt * M1 + m + 1) * 128],
                            rhs=h0_fm[:, kt * BLK : (kt + 1) * BLK],
                            start=(kt == 0), stop=(kt == KT1 - 1),
                        )
                    nc.scalar.activation(h1_fm[:, m * BLK : (m + 1) * BLK],
                                         hp[:], AF.Relu,
                                         bias=b1_sb[:, m : m + 1], scale=1.0)

            def emit_final(blk):
                """r = h1 . lin_w_b ; p = sigmoid(alpha + r + c0)."""
                rp_all = pss.tile([128, 256], dt.float32, space="PSUM",
                                  tag="s")
                for c in range(NCHUNK):
                    for kt in range(M1):
                        nc.tensor.matmul(
                            rp_all[:, c * 64 : c * 64 + 1],
                            lhsT=h1_fm[:, kt * BLK + c * 128 : kt * BLK + (c + 1) * 128],
                            rhs=lw2_sb[:, kt : kt + 1],
                            start=(kt == 0), stop=(kt == M1 - 1),
                        )
                    al = als.pop((blk, c))
                    t2 = scr.tile([128, 1], dt.float32, tag="t2")
                    nc.vector.tensor_tensor(t2[:], al[:],
                                            rp_all[:, c * 64 : c * 64 + 1],
                                            op=OP.add)
                    nc.scalar.activation(
                        p_sb[:, blk * NCHUNK + c : blk * NCHUNK + c + 1],
                        t2[:], AF.Sigmoid, bias=float(c0), scale=1.0)

            # ================= schedule =================
            if ablate == "gather":
                # gathers + merges only; p = 0.5 everywhere
                nc.vector.memset(p_sb[:], 0.0)
                for blk in range(NBLK):
                    emit_idx_dma(blk)
                for blk in range(NBLK):
                    emit_gathers(blk)
                    emit_merges(blk)
            elif ablate == "dumpflat":
                nc.vector.memset(p_sb[:], 0.0)
                for blk in range(NBLK):
                    emit_idx_dma(blk)
                for blk in range(NBLK):
                    emit_gathers(blk)
                    emit_merges(blk)
                    nc.sync.dma_start(fdump_d[blk, :, :],
                                      flat_fm[blk % 3][:])
            else:
                for blk in range(min(2, NBLK)):
                    emit_idx_dma(blk)
                emit_gathers(0)
                emit_merges(0)
                emit_gathers(1)
                emit_ctrl(0)
                emit_topk(0)
                emit_merges(1)
                emit_mask_transpose(0)

                for blk in range(NBLK):
                    if blk + 2 < NBLK:
                        emit_idx_dma(blk + 2)
                        emit_gathers(blk + 2)
                    emit_expand_x0(blk)
                    if blk + 1 < NBLK:
                        emit_ctrl(blk + 1)
                        emit_topk(blk + 1)
                    u_sb = emit_mlp0_u(blk)
                    if blk + 1 < NBLK:
                        emit_mask_transpose(blk + 1)
                    if ablate != "alpha":
                        emit_u_alpha(blk, u_sb)
                    else:
                        for c in range(NCHUNK):
                            al = scr.tile([128, 1], dt.float32, tag="al")
                            nc.vector.memset(al[:], 0.0)
                            als[(blk, c)] = al
                    emit_mlp1(blk)
                    emit_final(blk)
                    if blk + 2 < NBLK:
                        emit_merges(blk + 2)

            # ---- transpose p [128, 16] -> [16, 128] and store ----
            ptp = pss.tile([128, 256], dt.float32, space="PSUM", tag="s")
            nc.tensor.transpose(out=ptp[: BPC // 128, :128], in_=p_sb[:],
                                identity=idf[:])
            pout = cpool.tile([BPC // 128, 128], dt.float32)
            nc.vector.tensor_copy(pout[:], ptp[: BPC // 128, :128])
            nc.sync.dma_start(out_d[:].rearrange("(a b) -> a b", b=128),
                              pout[:])

    nc.compile()
    return nc


def _prep_host(inputs):
    """Host-side preprocessing -> per-core input maps."""
    x = np.asarray(inputs["x"]).astype(np.int64)
    tab = np.asarray(inputs["emb_table"], dtype=np.float32)
    k = int(np.asarray(inputs["k"]))

    s_f = (np.asarray(inputs["bn_gamma"], np.float64)
           / np.sqrt(np.asarray(inputs["bn_var"], np.float64) + EPS))
    t_f = np.asarray(inputs["bn_beta"], np.float64) - np.asarray(
        inputs["bn_mean"], np.float64) * s_f
    tab_bn = (tab.astype(np.float64) * np.repeat(s_f, VOCAB)[:, None]
              + np.repeat(t_f, VOCAB)[:, None])
    tab_h = np.zeros((F * VOCAB, 128), bf16)
    tab_h[:, :E] = tab_bn.astype(bf16)

    # controller weights, padded D 2496 -> 2560 with bias as ones-feature row
    wc = np.zeros((KT * 128, F), np.float32)
    wc[:D] = np.asarray(inputs["ctrl_w"], np.float32)
    wc[D] = np.asarray(inputs["ctrl_b"], np.float32)
    wc_h = np.ascontiguousarray(
        wc.reshape(KT, 128, F).transpose(1, 0, 2).reshape(128, KT * F)).astype(bf16)

    # MLP0 with BN scale folded into columns
    g0 = (np.asarray(inputs["mlp_g0"], np.float64)
          / np.sqrt(np.asarray(inputs["mlp_v0"], np.float64) + EPS))
    w0 = np.zeros((KT * 128, H0), np.float32)
    w0[:D] = np.asarray(inputs["mlp_w0"], np.float32) * g0[None, :].astype(np.float32)
    b0 = ((np.asarray(inputs["mlp_b0"], np.float64)
           - np.asarray(inputs["mlp_m0"], np.float64)) * g0
          + np.asarray(inputs["mlp_be0"], np.float64)).astype(np.float32)
    w0_h = np.ascontiguousarray(
        w0.reshape(KT, 128, M0, 128).transpose(1, 0, 2, 3)
        .reshape(128, KT * M0 * 128)).astype(bf16)
    b0_h = np.ascontiguousarray(b0.reshape(M0, 128).T)

    g1 = (np.asarray(inputs["mlp_g1"], np.float64)
          / np.sqrt(np.asarray(inputs["mlp_v1"], np.float64) + EPS))
    w1 = np.asarray(inputs["mlp_w1"], np.float32) * g1[None, :].astype(np.float32)
    b1 = ((np.asarray(inputs["mlp_b1"], np.float64)
           - np.asarray(inputs["mlp_m1"], np.float64)) * g1
          + np.asarray(inputs["mlp_be1"], np.float64)).astype(np.float32)
    w1_h = np.ascontiguousarray(
        w1.reshape(KT1, 128, M1, 128).transpose(1, 0, 2, 3)
        .reshape(128, KT1 * M1 * 128)).astype(bf16)
    b1_h = np.ascontiguousarray(b1.reshape(M1, 128).T)

    # U weights: 6 cross rows + lin_w[:D], padded, laid out per k-tile
    cross_w = np.asarray(inputs["cross_w"], np.float32)
    cross_b = np.asarray(inputs["cross_b"], np.float64)
    lin_w = np.asarray(inputs["lin_w"], np.float32)
    wu = np.zeros((KT * 128, 7), np.float32)
    wu[:D, :L] = cross_w.T
    wu[:D, 6] = lin_w[:D]
    wu_h = np.ascontiguousarray(
        wu.reshape(KT, 128, 7).transpose(1, 0, 2).reshape(128, KT * 7)).astype(bf16)

    # expand matrix S [F, KT*128]
    s = np.zeros((F, KT * 128), np.float32)
    feat = np.arange(KT * 128)
    valid = feat < D
    s[feat[valid] // E, feat[valid]] = 1.0
    s_h = s.astype(bf16)

    lw2_h = np.ascontiguousarray(lin_w[D:].reshape(M1, 128).T).astype(bf16)

    # cross-collapse constants: v_l = beta_l . w_l ; c0 = beta_6 . lin_w_a + b
    beta = np.zeros(D, np.float64)
    v = np.zeros(L, np.float64)
    for l in range(L):
        v[l] = beta @ cross_w[l].astype(np.float64)
        beta = beta + cross_b[l]
    c0 = float(beta @ lin_w[:D].astype(np.float64)
               + float(np.asarray(inputs["lin_b"]).ravel()[0]))
    v_consts = tuple(float(t) for t in v)

    def wrap16(jj):
        # idx j at [j % 16, j // 16], replicated over 8 partition groups
        w = jj.reshape(-1, 16).T.astype(np.int16)  # [16, n/16]
        return np.tile(w, (8, 1))  # [128, n/16]

    in_maps = []
    for ci in range(NCORES):
        xs = x[ci * BPC : (ci + 1) * BPC]  # [2048, 39]
        idxs = np.zeros((NBLK, 128, IDXW_TOT), np.int16)
        for blk in range(NBLK):
            rows = xs[blk * BLK : (blk + 1) * BLK]  # [512, 39]
            for g in range(KT):  # even fields (direct), slot g
                f = min(2 * g, 38)
                idxs[blk, :, g * IDXW_F : (g + 1) * IDXW_F] = \
                    wrap16(rows[:, f].copy())
            for g in range(19):  # odd fields (scratch), slot 20+g
                idxs[blk, :, (KT + g) * IDXW_F : (KT + g + 1) * IDXW_F] = \
                    wrap16(rows[:, 2 * g + 1].copy())
        in_maps.append({
            "idxs": idxs,
            "tab": tab_h,
            "wc": wc_h,
            "w0": w0_h,
            "w1": w1_h,
            "wu": wu_h,
            "s": s_h,
            "lw2": lw2_h,
            "b0": b0_h,
            "b1": b1_h,
        })
    return in_maps, k, v_consts, c0


def _scheduled_gather_queues(nc):
    """Walk the scheduled program; return [(key, ordinal, queue)] for
    every dma_gather, in scheduled (program) order."""
    out = []
    cnt = 0
    for f in nc.m.functions:
        for bb in f.blocks:
            for inst in bb.instructions:
                if type(inst).__name__ == "InstDMAGatherAnt":
                    key = nc._gather_insts.get(inst.name)
                    out.append((key, cnt, inst.queue_num))
                    cnt += 1
    return out


def _build_aligned(k, v_consts, c0, ablate=None):
    """Build, then verify the SWDGE queue assignment is consistent with
    the scheduler's mod-8 DMASW semaphore rotation (sem lane = scheduled
    ordinal % 8, each lane locked to one queue). If not, rebuild with
    queue = scheduled ordinal % NQ (fixpoint, few iterations)."""
    queue_map = {}
    for attempt in range(4):
        nc = _build(k, v_consts, c0, queue_map, ablate=ablate)
        sched = _scheduled_gather_queues(nc)
        lane_lock = {}
        ok = True
        for key, ordinal, q in sched:
            lane = ordinal % 8
            if lane_lock.setdefault(lane, q) != q:
                ok = False
        if ok:
            return nc
        new_map = {key: ordinal % NQ for key, ordinal, q in sched
                   if key is not None}
        if new_map == queue_map:
            return nc  # schedule oscillates; give up realigning
        queue_map = new_map
    return nc


def _get_nc(k, v_consts, c0):
    key = (k, v_consts, c0)
    if key not in _CACHE:
        _CACHE[key] = _build_aligned(k, v_consts, c0)
    return _CACHE[key]


def kernel(**inputs) -> np.ndarray:
    in_maps, k, v_consts, c0 = _prep_host(inputs)
    nc = _get_nc(k, v_consts, c0)
    res = run_bass_kernel_spmd(nc, in_maps, core_ids=list(range(NCORES)))
    out = np.concatenate([res.results[i]["out"] for i in range(NCORES)])
    return out.astype(np.float32)


def run_traced(**inputs):
    """Like kernel() but with tracing enabled; returns (out, results)."""
    in_maps, k, v_consts, c0 = _prep_host(inputs)
    nc = _get_nc(k, v_consts, c0)
    res = run_bass_kernel_spmd(nc, in_maps, core_ids=list(range(NCORES)),
                               trace=True)
    out = np.concatenate([res.results[i]["out"] for i in range(NCORES)])
    return out.astype(np.float32), res


# revision 36
# speedup vs baseline: 1.0444x; 1.0444x over previous
"""Trainium2 Bass kernel for nn_DeepCrossNetworkModel_Controller_hard.

Model: per-field embedding gather -> BatchNorm1d(F) (eval) -> controller
linear + softmax over fields -> top-k mask (renormalized) -> CrossNetwork(6)
+ MLP(2496->1024->512, BN+ReLU) -> concat -> linear -> sigmoid.

Strategy (data-parallel over 8 NeuronCores, 2048 rows each):
 - BN folded into the embedding table on host; table stored bf16 padded to
   128 elems/row so dma_gather(transpose=True) lands embeddings
   FEATURE-major directly (no PE transposes at all).
 - top-k of softmax + renormalize == softmax restricted to top-k logits.
 - CrossNetwork collapses algebraically: on device only U = x0 @ wu
   (7 cols, folded into the MLP0 m-tile loop) + a scalar recursion.
 - Software-pipelined: gathers run two blocks ahead, controller/top-k of
   block b+1 execute under MLP0 of block b, so the PE never idles.
"""

import sys

if "/opt/trn_rl_repo" not in sys.path:
    sys.path.insert(0, "/opt/trn_rl_repo")

import ml_dtypes
import numpy as np

import concourse.bass as bass
import concourse.bacc as bacc
import concourse.mybir as mybir
import concourse.tile as tile
from concourse.bass_utils import run_bass_kernel_spmd
from concourse.masks import make_identity

# Problem constants (hardcoded per spec).
B, F, E, L = 16384, 39, 64, 6
VOCAB = 10000
D = F * E  # 2496
H0, H1 = 1024, 512
EPS = 1e-5
NCORES = 8
BPC = B // NCORES      # 2048 rows per core
BLK = 512              # batch block
NBLK = BPC // BLK      # 4
NCHUNK = BLK // 128    # 4 chunks of 128 rows per block
KT = 20                # feature k-tiles of 128 (D padded 2496 -> 2560)
M0 = H0 // 128         # 8
M1 = H1 // 128         # 4
KT1 = H0 // 128        # 8
NQ = 4                 # SWDGE queues
# idx widths (wrapped-by-16 free dims): 39 single-field gathers of 512
# (transpose-mode dma_gather hangs on HW above 512 idxs per call)
IDXW_F = 512 // 16       # 32
IDXW_TOT = 39 * IDXW_F   # 1248: 20 even-field (direct) + 19 odd (scratch)

dt = mybir.dt
AF = mybir.ActivationFunctionType
OP = mybir.AluOpType
bf16 = ml_dtypes.bfloat16

_CACHE = {}


def _build(k, v_consts, c0, queue_map=None, ablate=None, all_scratch=False,
           sp=True):
    """Build the per-core SPMD bass module.

    queue_map: optional {(blk, g): queue_num} overriding the default
    g % NQ assignment, used to realign SWDGE queues with the
    scheduler's mod-8 DMASW semaphore rotation (see _build_aligned).
    """
    queue_map = queue_map or {}
    gather_insts = {}
    nc = bacc.Bacc("TRN2", target_bir_lowering=False, debug=False,
                   num_devices=NCORES, num_swdge_queues=NQ)
    nc._gather_insts = gather_insts

    idxs_d = nc.declare_dram_parameter("idxs", [NBLK, 128, IDXW_TOT], dt.int16, isOutput=False)
    tab_d = nc.declare_dram_parameter("tab", [F * VOCAB, 128], dt.bfloat16, isOutput=False)
    wc_d = nc.declare_dram_parameter("wc", [128, KT * F], dt.bfloat16, isOutput=False)
    w0_d = nc.declare_dram_parameter("w0", [128, KT * M0 * 128], dt.bfloat16, isOutput=False)
    w1_d = nc.declare_dram_parameter("w1", [128, KT1 * M1 * 128], dt.bfloat16, isOutput=False)
    wu_d = nc.declare_dram_parameter("wu", [128, KT * 7], dt.bfloat16, isOutput=False)
    s_d = nc.declare_dram_parameter("s", [F, KT * 128], dt.bfloat16, isOutput=False)
    lw2_d = nc.declare_dram_parameter("lw2", [128, M1], dt.bfloat16, isOutput=False)
    b0_d = nc.declare_dram_parameter("b0", [128, M0], dt.float32, isOutput=False)
    b1_d = nc.declare_dram_parameter("b1", [128, M1], dt.float32, isOutput=False)
    out_d = nc.declare_dram_parameter("out", [BPC], dt.float32, isOutput=True)
    if ablate == "dumpflat":
        fdump_d = nc.declare_dram_parameter(
            "fdump", [NBLK, 128, KT * BLK], dt.bfloat16, isOutput=True)

    rounds = -(-k // 8)  # ceil(k/8) match_replace rounds

    with tile.TileContext(nc) as tc:
        with (
            tc.tile_pool(name="const", bufs=1) as cpool,
            tc.tile_pool(name="flat", bufs=1) as flatp,
            tc.tile_pool(name="big", bufs=1) as bigp,
            tc.tile_pool(name="msk", bufs=2) as mskp,
            tc.tile_pool(name="gat", bufs=20) as gatp,
            tc.tile_pool(name="idx", bufs=3) as idxp,
            tc.tile_pool(name="scr", bufs=8) as scr,
            # PSUM slots are bank-granular (8 banks x 2KB/partition):
            # psb "b" x3 (hp/ex/up) + psz "z" x2 + pss "s" x2 + pmt "t" x1
            tc.tile_pool(name="psb", bufs=3, space="PSUM") as psb,
            tc.tile_pool(name="psz", bufs=2, space="PSUM") as psz,
            tc.tile_pool(name="pss", bufs=2, space="PSUM") as pss,
            tc.tile_pool(name="pmt", bufs=1, space="PSUM") as pmt,
        ):
            # ---- identities + small constants first (cheap, unblock PE) ----
            idf = cpool.tile([128, 128], dt.float32)
            make_identity(nc, idf[:])
            idb = cpool.tile([128, 128], dt.bfloat16)
            make_identity(nc, idb[:])

            wc_sb = cpool.tile([128, KT * F], dt.bfloat16)
            nc.sync.dma_start(wc_sb[:], wc_d[:])
            s_sb = cpool.tile([F, KT * 128], dt.bfloat16)
            nc.sync.dma_start(s_sb[:], s_d[:])
            wu_sb = cpool.tile([128, KT * 7], dt.bfloat16)
            nc.sync.dma_start(wu_sb[:], wu_d[:])
            lw2_sb = cpool.tile([128, M1], dt.bfloat16)
            nc.sync.dma_start(lw2_sb[:], lw2_d[:])
            b0_sb = cpool.tile([128, M0], dt.float32)
            nc.sync.dma_start(b0_sb[:], b0_d[:])
            b1_sb = cpool.tile([128, M1], dt.float32)
            nc.sync.dma_start(b1_sb[:], b1_d[:])
            # big weights last, in pieces, so early DMA traffic unblocks first
            w1_sb = cpool.tile([128, KT1 * M1 * 128], dt.bfloat16)
            nc.sync.dma_start(w1_sb[:], w1_d[:])
            w0_sb = cpool.tile([128, KT * M0 * 128], dt.bfloat16)
            for q in range(4):
                qs = KT * M0 * 128 // 4
                nc.sync.dma_start(w0_sb[:, q * qs : (q + 1) * qs],
                                  w0_d[:, q * qs : (q + 1) * qs])

            # ---- persistent activations ----
            # flat/x0 triple buffer: landing b+2, ctrl reads b+1, x0 in b
            flat_fm = [flatp.tile([128, KT * BLK], dt.bfloat16, tag=f"f{i}",
                                  name=f"flat{i}")
                       for i in range(3)]
            h0_fm = bigp.tile([128, M0 * BLK], dt.bfloat16)
            h1_fm = bigp.tile([128, M1 * BLK], dt.bfloat16)
            p_sb = bigp.tile([128, BPC // 128], dt.float32)

            nreg = nc.gpsimd.to_reg(512)
            qcount = [0]  # global SWDGE queue round-robin

            idx_sb = {}
            gts = {}
            masks = {}
            zs = {}
            als = {}

            def emit_idx_dma(blk):
                t = idxp.tile([128, IDXW_TOT], dt.int16, tag="i")
                nc.sync.dma_start(t[:], idxs_d[blk, :, :])
                idx_sb[blk] = t

            def _gather(key, out_ap, lo, it, slot):
                inst = nc.gpsimd.dma_gather(
                    out_ap=out_ap,
                    in_ap=tab_d[lo : lo + VOCAB, :],
                    idxs_ap=it[:, slot * IDXW_F : (slot + 1) * IDXW_F],
                    num_idxs=512,
                    num_idxs_reg=nreg,
                    elem_size=128,
                    transpose=True,
                    single_packet=sp,
                    queue_num=queue_map.get(key, qcount[0] % NQ),
                )
                qcount[0] += 1
                gather_insts[inst.ins.name] = key

            def emit_gathers(blk):
                """39 single-field 512-idx gathers per block: even fields
                land directly in flat k-tile slots (with zeros on parts
                64:128), odd fields go to scratch for a DVE merge."""
                it = idx_sb.pop(blk)
                ft = flat_fm[blk % 3]
                for g in range(KT):
                    f = min(2 * g, 38)
                    if all_scratch:
                        etile = gatp.tile([128, BLK], dt.bfloat16, tag="e",
                                          name="etile")
                        _gather((blk, g, 0),
                                etile[:].rearrange("p (a n) -> p a n", a=1),
                                f * VOCAB, it, g)
                        gts[(blk, g, 0)] = etile
                    else:
                        _gather((blk, g, 0),
                                ft[:, g * BLK : (g + 1) * BLK].rearrange(
                                    "p (a n) -> p a n", a=1),
                                f * VOCAB, it, g)
                    if g < 19:
                        gtile = gatp.tile([128, BLK], dt.bfloat16, tag="g")
                        _gather((blk, g, 1),
                                gtile[:].rearrange("p (a n) -> p a n", a=1),
                                (2 * g + 1) * VOCAB, it, KT + g)
                        gts[(blk, g)] = gtile

            def emit_merges(blk):
                """SBUF->SBUF DMA moving odd-field scratch into flat upper
                partitions (keeps DVE free), plus the ones-row feature."""
                ft = flat_fm[blk % 3]
                for g in range(KT):
                    if all_scratch:
                        nc.sync.dma_start(
                            ft[0:128, g * BLK : (g + 1) * BLK],
                            gts.pop((blk, g, 0))[0:128, 0:BLK])
                for g in range(19):
                    gtile = gts.pop((blk, g))
                    nc.sync.dma_start(
                        ft[64:128, g * BLK : (g + 1) * BLK],
                        gtile[0:64, 0:BLK])
                nc.vector.memset(ft[64:65, 19 * BLK : 20 * BLK], 1.0)

            def emit_ctrl(blk):
                """Controller matmuls: z[chunk] = flat_chunk^T @ wc.
                All 4 chunks share one PSUM bank tile (col ranges)."""
                ft = flat_fm[blk % 3]
                z_all = psz.tile([128, 256], dt.float32, space="PSUM", tag="z")
                for c in range(NCHUNK):
                    for kt in range(KT):
                        nc.tensor.matmul(
                            z_all[:, c * 64 : c * 64 + F],
                            lhsT=ft[:, kt * BLK + c * 128 : kt * BLK + (c + 1) * 128],
                            rhs=wc_sb[:, kt * F : (kt + 1) * F],
                            start=(kt == 0), stop=(kt == KT - 1),
                        )
                zs[blk] = z_all

            def emit_topk(blk):
                """Top-k + renormalized softmax -> batch-major mask (bf16)."""
                z_all = zs.pop(blk)
                for c in range(NCHUNK):
                    z = z_all[:, c * 64 : (c + 1) * 64]
                    mx = scr.tile([128, 8], dt.float32, tag="mx")
                    nm = scr.tile([128, 1], dt.float32, tag="nm")
                    zap = scr.tile([128, F], dt.float32, tag="zap")
                    zap2 = scr.tile([128, F], dt.float32, tag="zap2")
                    esb = scr.tile([128, F], dt.float32, tag="esb")
                    ssum = scr.tile([128, 1], dt.float32, tag="ssum")
                    rcp = scr.tile([128, 1], dt.float32, tag="rcp")
                    mbm = scr.tile([128, F], dt.bfloat16, tag="mbm")
                    src = z[:, :F]
                    outs = [zap[:], zap2[:]]
                    for r in range(rounds):
                        nc.vector.max(out=mx[:], in_=src)
                        if r == 0:
                            nc.vector.tensor_scalar(
                                nm[:], mx[:, 0:1], -1.0, None, op0=OP.mult)
                        if r == rounds - 1 and k - 8 * r < 8:
                            nc.vector.memset(mx[:, k - 8 * r :], -1e30)
                        nc.vector.match_replace(
                            out=outs[r % 2], in_to_replace=mx[:],
                            in_values=src, imm_value=-1e30)
                        src = outs[r % 2]
                    zfin = outs[(rounds - 1) % 2]
                    nc.scalar.activation(esb[:], z[:, :F], AF.Exp,
                                         bias=nm[:, 0:1], scale=1.0)
                    nc.vector.tensor_scalar(zfin, zfin, -1e30, None,
                                            op0=OP.is_equal)
                    nc.vector.tensor_tensor(esb[:], esb[:], zfin, op=OP.mult)
                    nc.vector.reduce_sum(ssum[:], esb[:],
                                         axis=mybir.AxisListType.X)
                    nc.vector.reciprocal(rcp[:], ssum[:])
                    nc.vector.tensor_scalar(mbm[:], esb[:], rcp[:, 0:1],
                                            None, op0=OP.mult)
                    masks[(blk, c)] = mbm

            def emit_mask_transpose(blk):
                """Batch-major masks -> feature-major mask_fm [F, BLK]."""
                mt_fm = mskp.tile([F, BLK], dt.bfloat16, tag="m")
                mt = pmt.tile([128, BLK], dt.bfloat16, space="PSUM", tag="t")
                for c in range(NCHUNK):
                    mbm = masks.pop((blk, c))
                    nc.tensor.transpose(
                        out=mt[:F, c * 128 : (c + 1) * 128], in_=mbm[:],
                        identity=idb[:])
                    nc.vector.tensor_copy(
                        mt_fm[:, c * 128 : (c + 1) * 128],
                        mt[:F, c * 128 : (c + 1) * 128])
                masks[blk] = mt_fm

            def emit_expand_x0(blk):
                """Expand mask over features (PE) and apply to flat in
                place: x0 = flat * S^T mask."""
                ft = flat_fm[blk % 3]
                mt_fm = masks.pop(blk)
                for kt in range(KT):
                    ex = psb.tile([128, BLK], dt.float32, space="PSUM",
                                  tag="b")
                    nc.tensor.matmul(
                        ex[:], lhsT=s_sb[:, kt * 128 : (kt + 1) * 128],
                        rhs=mt_fm[:], start=True, stop=True)
                    nc.vector.tensor_tensor(
                        ft[:, kt * BLK : (kt + 1) * BLK],
                        ft[:, kt * BLK : (kt + 1) * BLK], ex[:],
                        op=OP.mult)

            def emit_mlp0_u(blk):
                """U (7-wide m-tile) first, then MLP0's 8 m-tiles."""
                ft = flat_fm[blk % 3]
                up = psb.tile([128, BLK], dt.float32, space="PSUM", tag="b")
                for kt in range(KT):
                    nc.tensor.matmul(
                        up[:7, :],
                        lhsT=wu_sb[:, kt * 7 : (kt + 1) * 7],
                        rhs=ft[:, kt * BLK : (kt + 1) * BLK],
                        start=(kt == 0), stop=(kt == KT - 1),
                    )
                u_sb = scr.tile([7, BLK], dt.float32, tag="usb", bufs=2)
                nc.vector.tensor_copy(u_sb[:], up[:7, :])
                for m in range(M0):
                    hp = psb.tile([128, BLK], dt.float32, space="PSUM",
                                  tag="b")
                    for kt in range(KT):
                        nc.tensor.matmul(
                            hp[:],
                            lhsT=w0_sb[:, (kt * M0 + m) * 128 : (kt * M0 + m + 1) * 128],
                            rhs=ft[:, kt * BLK : (kt + 1) * BLK],
                            start=(kt == 0), stop=(kt == KT - 1),
                        )
                    nc.scalar.activation(h0_fm[:, m * BLK : (m + 1) * BLK],
                                         hp[:], AF.Relu,
                                         bias=b0_sb[:, m : m + 1], scale=1.0)
                return u_sb

            def emit_u_alpha(blk, u_sb):
                """Transpose u to batch-major and run the cross-collapse
                scalar recursion per 128-row chunk."""
                ut_all = pss.tile([128, 256], dt.float32, space="PSUM",
                                  tag="s")
                for c in range(NCHUNK):
                    nc.tensor.transpose(
                        out=ut_all[:, c * 64 : c * 64 + 7],
                        in_=u_sb[:, c * 128 : (c + 1) * 128],
                        identity=idf[:7, :7],
                    )
                    usb = scr.tile([128, 8], dt.float32, tag="ut")
                    nc.vector.tensor_copy(usb[:, :7],
                                          ut_all[:, c * 64 : c * 64 + 7])
                    al = scr.tile([128, 1], dt.float32, tag="al")
                    t1 = scr.tile([128, 1], dt.float32, tag="t1")
                    nc.vector.tensor_scalar(al[:], usb[:, 0:1],
                                            1.0 + v_consts[0], None, op0=OP.add)
                    for l in range(1, L):
                        nc.vector.tensor_scalar(t1[:], usb[:, l : l + 1],
                                                1.0, None, op0=OP.add)
                        nc.vector.tensor_tensor(al[:], al[:], t1[:],
                                                op=OP.mult)
                        if v_consts[l] != 0.0:
                            nc.vector.tensor_scalar(al[:], al[:],
                                                    v_consts[l], None,
                                                    op0=OP.add)
                    nc.vector.tensor_tensor(al[:], al[:], usb[:, 6:7],
                                            op=OP.mult)
                    als[(blk, c)] = al

            def emit_mlp1(blk):
                for m in range(M1):
                    hp = psb.tile([128, BLK], dt.float32, space="PSUM",
                                  tag="b")
                    for kt in range(KT1):
                        nc.tensor.matmul(
                            hp[:],
                            lhsT=w1_sb[:, (kt * M1 + m) * 128 : (kt * M1 + m + 1) * 128],
                            rhs=h0_fm[:, kt * BLK : (kt + 1) * BLK],
                            start=(kt == 0), stop=(kt == KT1 - 1),
                        )
                    nc.scalar.activation(h1_fm[:, m * BLK : (m + 1) * BLK],
                                         hp[:], AF.Relu,
                                         bias=b1_sb[:, m : m + 1], scale=1.0)

            def emit_final(blk):
                """r = h1 . lin_w_b ; p = sigmoid(alpha + r + c0)."""
                rp_all = pss.tile([128, 256], dt.float32, space="PSUM",
                                  tag="s")
                for c in range(NCHUNK):
                    for kt in range(M1):
                        nc.tensor.matmul(
                            rp_all[:, c * 64 : c * 64 + 1],
                            lhsT=h1_fm[:, kt * BLK + c * 128 : kt * BLK + (c + 1) * 128],
                            rhs=lw2_sb[:, kt : kt + 1],
                            start=(kt == 0), stop=(kt == M1 - 1),
                        )
                    al = als.pop((blk, c))
                    t2 = scr.tile([128, 1], dt.float32, tag="t2")
                    nc.vector.tensor_tensor(t2[:], al[:],
                                            rp_all[:, c * 64 : c * 64 + 1],
                                            op=OP.add)
                    nc.scalar.activation(
                        p_sb[:, blk * NCHUNK + c : blk * NCHUNK + c + 1],
                        t2[:], AF.Sigmoid, bias=float(c0), scale=1.0)

            # ================= schedule =================
            if ablate == "gather":
                # gathers + merges only; p = 0.5 everywhere
                nc.vector.memset(p_sb[:], 0.0)
                for blk in range(NBLK):
                    emit_idx_dma(blk)
                for blk in range(NBLK):
                    emit_gathers(blk)
                    emit_merges(blk)
            elif ablate == "dumpflat":
                nc.vector.memset(p_sb[:], 0.0)
                for blk in range(NBLK):
                    emit_idx_dma(blk)
                for blk in range(NBLK):
                    emit_gathers(blk)
                    emit_merges(blk)
                    nc.sync.dma_start(fdump_d[blk, :, :],
                                      flat_fm[blk % 3][:])
            else:
                for blk in range(min(2, NBLK)):
                    emit_idx_dma(blk)
                emit_gathers(0)
                emit_merges(0)
                emit_gathers(1)
                emit_ctrl(0)
                emit_topk(0)
                emit_merges(1)
                emit_mask_transpose(0)

                for blk in range(NBLK):
                    if blk + 2 < NBLK:
                        emit_idx_dma(blk + 2)
                        emit_gathers(blk + 2)
                    emit_expand_x0(blk)
                    if blk + 1 < NBLK:
                        emit_ctrl(blk + 1)
                        emit_topk(blk + 1)
                    u_sb = emit_mlp0_u(blk)
                    if blk + 1 < NBLK:
                        emit_mask_transpose(blk + 1)
                    if ablate != "alpha":
                        emit_u_alpha(blk, u_sb)
                    else:
                        for c in range(NCHUNK):
                            al = scr.tile([128, 1], dt.float32, tag="al")
                            nc.vector.memset(al[:], 0.0)
                            als[(blk, c)] = al
                    emit_mlp1(blk)
                    emit_final(blk)
                    if blk + 2 < NBLK:
                        emit_merges(blk + 2)

            # ---- transpose p [128, 16] -> [16, 128] and store ----
            ptp = pss.tile([128, 256], dt.float32, space="PSUM", tag="s")
            nc.tensor.transpose(out=ptp[: BPC // 128, :128], in_=p_sb[:],
                                identity=idf[:])
            pout = cpool.tile([BPC // 128, 128], dt.float32)
            nc.vector.tensor_copy(pout[:], ptp[: BPC // 128, :128])
            nc.sync.dma_start(out_d[:].rearrange("(a b) -> a b", b=128),
                              pout[:])

    nc.compile()
    return nc


def _prep_host(inputs):
    """Host-side preprocessing -> per-core input maps."""
    x = np.asarray(inputs["x"]).astype(np.int64)
    tab = np.asarray(inputs["emb_table"], dtype=np.float32)
    k = int(np.asarray(inputs["k"]))

    s_f = (np.asarray(inputs["bn_gamma"], np.float64)
           / np.sqrt(np.asarray(inputs["bn_var"], np.float64) + EPS))
    t_f = np.asarray(inputs["bn_beta"], np.float64) - np.asarray(
        inputs["bn_mean"], np.float64) * s_f
    tab_bn = (tab.astype(np.float64) * np.repeat(s_f, VOCAB)[:, None]
              + np.repeat(t_f, VOCAB)[:, None])
    tab_h = np.zeros((F * VOCAB, 128), bf16)
    tab_h[:, :E] = tab_bn.astype(bf16)

    # controller weights, padded D 2496 -> 2560 with bias as ones-feature row
    wc = np.zeros((KT * 128, F), np.float32)
    wc[:D] = np.asarray(inputs["ctrl_w"], np.float32)
    wc[D] = np.asarray(inputs["ctrl_b"], np.float32)
    wc_h = np.ascontiguousarray(
        wc.reshape(KT, 128, F).transpose(1, 0, 2).reshape(128, KT * F)).astype(bf16)

    # MLP0 with BN scale folded into columns
    g0 = (np.asarray(inputs["mlp_g0"], np.float64)
          / np.sqrt(np.asarray(inputs["mlp_v0"], np.float64) + EPS))
    w0 = np.zeros((KT * 128, H0), np.float32)
    w0[:D] = np.asarray(inputs["mlp_w0"], np.float32) * g0[None, :].astype(np.float32)
    b0 = ((np.asarray(inputs["mlp_b0"], np.float64)
           - np.asarray(inputs["mlp_m0"], np.float64)) * g0
          + np.asarray(inputs["mlp_be0"], np.float64)).astype(np.float32)
    w0_h = np.ascontiguousarray(
        w0.reshape(KT, 128, M0, 128).transpose(1, 0, 2, 3)
        .reshape(128, KT * M0 * 128)).astype(bf16)
    b0_h = np.ascontiguousarray(b0.reshape(M0, 128).T)

    g1 = (np.asarray(inputs["mlp_g1"], np.float64)
          / np.sqrt(np.asarray(inputs["mlp_v1"], np.float64) + EPS))
    w1 = np.asarray(inputs["mlp_w1"], np.float32) * g1[None, :].astype(np.float32)
    b1 = ((np.asarray(inputs["mlp_b1"], np.float64)
           - np.asarray(inputs["mlp_m1"], np.float64)) * g1
          + np.asarray(inputs["mlp_be1"], np.float64)).astype(np.float32)
    w1_h = np.ascontiguousarray(
        w1.reshape(KT1, 128, M1, 128).transpose(1, 0, 2, 3)
        .reshape(128, KT1 * M1 * 128)).astype(bf16)
    b1_h = np.ascontiguousarray(b1.reshape(M1, 128).T)

    # U weights: 6 cross rows + lin_w[:D], padded, laid out per k-tile
    cross_w = np.asarray(inputs["cross_w"], np.float32)
    cross_b = np.asarray(inputs["cross_b"], np.float64)
    lin_w = np.asarray(inputs["lin_w"], np.float32)
    wu = np.zeros((KT * 128, 7), np.float32)
    wu[:D, :L] = cross_w.T
    wu[:D, 6] = lin_w[:D]
    wu_h = np.ascontiguousarray(
        wu.reshape(KT, 128, 7).transpose(1, 0, 2).reshape(128, KT * 7)).astype(bf16)

    # expand matrix S [F, KT*128]
    s = np.zeros((F, KT * 128), np.float32)
    feat = np.arange(KT * 128)
    valid = feat < D
    s[feat[valid] // E, feat[valid]] = 1.0
    s_h = s.astype(bf16)

    lw2_h = np.ascontiguousarray(lin_w[D:].reshape(M1, 128).T).astype(bf16)

    # cross-collapse constants: v_l = beta_l . w_l ; c0 = beta_6 . lin_w_a + b
    beta = np.zeros(D, np.float64)
    v = np.zeros(L, np.float64)
    for l in range(L):
        v[l] = beta @ cross_w[l].astype(np.float64)
        beta = beta + cross_b[l]
    c0 = float(beta @ lin_w[:D].astype(np.float64)
               + float(np.asarray(inputs["lin_b"]).ravel()[0]))
    v_consts = tuple(float(t) for t in v)

    def wrap16(jj):
        # idx j at [j % 16, j // 16], replicated over 8 partition groups
        w = jj.reshape(-1, 16).T.astype(np.int16)  # [16, n/16]
        return np.tile(w, (8, 1))  # [128, n/16]

    in_maps = []
    for ci in range(NCORES):
        xs = x[ci * BPC : (ci + 1) * BPC]  # [2048, 39]
        idxs = np.zeros((NBLK, 128, IDXW_TOT), np.int16)
        for blk in range(NBLK):
            rows = xs[blk * BLK : (blk + 1) * BLK]  # [512, 39]
            for g in range(KT):  # even fields (direct), slot g
                f = min(2 * g, 38)
                idxs[blk, :, g * IDXW_F : (g + 1) * IDXW_F] = \
                    wrap16(rows[:, f].copy())
            for g in range(19):  # odd fields (scratch), slot 20+g
                idxs[blk, :, (KT + g) * IDXW_F : (KT + g + 1) * IDXW_F] = \
                    wrap16(rows[:, 2 * g + 1].copy())
        in_maps.append({
            "idxs": idxs,
            "tab": tab_h,
            "wc": wc_h,
            "w0": w0_h,
            "w1": w1_h,
            "wu": wu_h,
            "s": s_h,
            "lw2": lw2_h,
            "b0": b0_h,
            "b1": b1_h,
        })
    return in_maps, k, v_consts, c0


def _scheduled_gather_queues(nc):
    """Walk the scheduled program; return [(key, ordinal, queue)] for
    every dma_gather, in scheduled (program) order."""
    out = []
    cnt = 0
    for f in nc.m.functions:
        for bb in f.blocks:
            for inst in bb.instructions:
                if type(inst).__name__ == "InstDMAGatherAnt":
                    key = nc._gather_insts.get(inst.name)
                    out.append((key, cnt, inst.queue_num))
                    cnt += 1
    return out


def _build_aligned(k, v_consts, c0, ablate=None):
    """Build, then verify the SWDGE queue assignment is consistent with
    the scheduler's mod-8 DMASW semaphore rotation (sem lane = scheduled
    ordinal % 8, each lane locked to one queue). If not, rebuild with
    queue = scheduled ordinal % NQ (fixpoint, few iterations)."""
    queue_map = {}
    for attempt in range(4):
        nc = _build(k, v_consts, c0, queue_map, ablate=ablate)
        sched = _scheduled_gather_queues(nc)
        lane_lock = {}
        ok = True
        for key, ordinal, q in sched:
            lane = ordinal % 8
            if lane_lock.setdefault(lane, q) != q:
                ok = False
        if ok:
            return nc
        new_map = {key: ordinal % NQ for key, ordinal, q in sched
                   if key is not None}
        if new_map == queue_map:
            return nc  # schedule oscillates; give up realigning
        queue_map = new_map
    return nc


def _get_nc(k, v_consts, c0):
    key = (k, v_consts, c0)
    if key not in _CACHE:
        _CACHE[key] = _build_aligned(k, v_consts, c0)
    return _CACHE[key]


def kernel(**inputs) -> np.ndarray:
    in_maps, k, v_consts, c0 = _prep_host(inputs)
    nc = _get_nc(k, v_consts, c0)
    res = run_bass_kernel_spmd(nc, in_maps, core_ids=list(range(NCORES)))
    out = np.concatenate([res.results[i]["out"] for i in range(NCORES)])
    return out.astype(np.float32)


def run_traced(**inputs):
    """Like kernel() but with tracing enabled; returns (out, results)."""
    in_maps, k, v_consts, c0 = _prep_host(inputs)
    nc = _get_nc(k, v_consts, c0)
    res = run_bass_kernel_spmd(nc, in_maps, core_ids=list(range(NCORES)),
                               trace=True)
    out = np.concatenate([res.results[i]["out"] for i in range(NCORES)])
    return out.astype(np.float32), res
